# revision 29
# baseline (speedup 1.0000x reference)
"""Causal self-attention (B=2, T=2048, C=2048, H=16) on 8 Trainium2 NeuronCores.

Sharding: tensor-parallel over heads x data-parallel over batch.
core = b*4 + hg handles batch b and head-group hg (4 heads = 512 channels).

Per-core device program (single SPMD Bass/Tile kernel, float32r matmuls):
  1. QKV: stream x^T once; per 512-wide t-chunk compute V natural [T,512]
     (lhsT = x^T tile) and QK^T channel-major [1024,T] (lhsT = W_qk tile),
     sharing the same SBUF x tiles across both + a second QK pass.
  2. per head h, q-chunk qc: S^T[kk,q] = K_h^T-tile.T @ Q_h^T-chunk for the
     causal kk-tiles only; additive -1e9 mask folded into the PSUM
     accumulation via an identity matmul on diagonal tiles;
     P = exp(scale*S^T) on ACT (no max-subtract needed: |logits| < ~10);
     O^T[d,q] accumulates V-tile.T @ P; denominators via all-ones matmul
     (partition reduction + broadcast in one op); normalize on DVE.
  3. y_partial = O @ W_p[hg rows] -> [T, C]

Host: shards/transposes inputs, gathers y = sum over hg of partials + b_proj,
cache columns from K^T (transposed on host) and V.
"""
import sys
import numpy as np

if "/opt/trn_rl_repo" not in sys.path:
    sys.path.insert(0, "/opt/trn_rl_repo")

B, T, C = 2, 2048, 2048
H = 16
DH = 128
NCORES = 8
HG = 4                # head-groups (tensor-parallel degree)
HL = H // HG          # heads per core = 4
DQ = HL * DH          # local q/k/v width = 512
CT = C // 128         # contraction tiles = 16
TT = T // 128         # t tiles = 16
TC4 = T // 512        # t chunks of 512 = 4
SM_SCALE = 1.0 / float(np.sqrt(DH))

_CACHE = {}


def _build():
    import concourse.bass as bass
    import concourse.tile as tile
    from concourse import bacc, mybir

    f32 = mybir.dt.float32
    f32r = mybir.dt.float32r
    Exp = mybir.ActivationFunctionType.Exp
    Ident = mybir.ActivationFunctionType.Identity
    MUL = mybir.AluOpType.mult
    ADD = mybir.AluOpType.add

    nc = bacc.Bacc("TRN2", target_bir_lowering=False, debug=False,
                   enable_asserts=True, num_devices=NCORES)

    xT = nc.dram_tensor("xT", [C, T], f32r, kind="ExternalInput").ap()
    w_qk = nc.dram_tensor("w_qk", [C, 2 * DQ], f32r, kind="ExternalInput").ap()
    w_v = nc.dram_tensor("w_v", [C, DQ], f32r, kind="ExternalInput").ap()
    w_p = nc.dram_tensor("w_p", [128, HL, C], f32r, kind="ExternalInput").ap()
    b_qk = nc.dram_tensor("b_qk", [128, 2 * HL], f32, kind="ExternalInput").ap()
    b_v = nc.dram_tensor("b_v", [128, DQ], f32, kind="ExternalInput").ap()
    masks = nc.dram_tensor("masks", [128, 4 * 512], f32r, kind="ExternalInput").ap()
    ones_d = nc.dram_tensor("ones_d", [128, 128], f32r, kind="ExternalInput").ap()
    ident_d = nc.dram_tensor("ident_d", [128, 128], f32r, kind="ExternalInput").ap()

    y_p = nc.dram_tensor("y_p", [T, C], f32, kind="ExternalOutput").ap()
    kT_out = nc.dram_tensor("kT_out", [DQ, T], f32r, kind="ExternalOutput").ap()
    v_out = nc.dram_tensor("v_out", [T, DQ], f32r, kind="ExternalOutput").ap()

    with tile.TileContext(nc) as tc:
        with tc.tile_pool(name="consts", bufs=1) as consts, \
             tc.tile_pool(name="persist", bufs=1) as persist:

            b_qk_sb = consts.tile([128, 2 * HL], f32)
            b_v_sb = consts.tile([128, DQ], f32)

            qk_sb = persist.tile([128, 2 * HL, T], f32r)    # [d, ch_blk, t] q blocks 0..3, k blocks 4..7
            v_sb = persist.tile([128, TT, DQ], f32r)        # [t_in, t_tile, vch]

            # ------------- Phase QKV (unified, x streamed once) -------------
            # Per 512-wide t-chunk tcx: stream the 16 x^T tiles [128c, 512t];
            # each tile is rhs for QK-blocks (ch-major out) and lhsT for V
            # (natural out).  PSUM: pass A = QK blks 0-3 (4) + V t-subtiles (4);
            # pass B = QK blks 4-7 reuses freed banks.
            with tc.tile_pool(name="ps8", bufs=8, space="PSUM") as ps8, \
                 tc.tile_pool(name="wqk", bufs=1) as wqkp:
                w_qk_sb = wqkp.tile([128, CT, 2 * DQ], f32r)
                with tc.tile_pool(name="xq", bufs=4) as xqp, \
                     tc.tile_pool(name="wv", bufs=3) as wvp:
                    for tcx in range(TC4):
                        psq = []
                        for blk in range(HL):
                            p = ps8.tile([128, 512], f32, tag="ps512", name=f"psq_{tcx}_{blk}")
                            psq.append(p)
                        psv = []
                        for ts_ in range(4):
                            p = ps8.tile([128, DQ], f32, tag="ps512", name=f"psv_{tcx}_{ts_}")
                            psv.append(p)
                        xts = []
                        for cg in range(4):
                            first = tcx == 0 and cg == 0
                            xq4 = xqp.tile([128, 4, 512], f32r, tag="xq", name=f"xqa_{tcx}_{cg}")
                            src_x = xT[cg * 512:(cg + 1) * 512,
                                       tcx * 512:(tcx + 1) * 512].rearrange("(o p) t -> p o t", p=128)
                            wv2a = wvp.tile([128, 2, DQ], f32r, tag="wv", name=f"wva_{tcx}_{cg}")
                            src_wv = w_v[cg * 512:cg * 512 + 256, :].rearrange("(o p) n -> p o n", p=128)
                            if first:
                                # minimal critical set for the first matmuls:
                                # 256KB wv (gpsimd) + 256KB x (sync); everything
                                # else streams behind it
                                nc.gpsimd.dma_start(wv2a[:, 0:1], src_wv[:, 0:1])
                                nc.sync.dma_start(xq4[:, 0:1], src_x[:, 0:1])
                                nc.gpsimd.dma_start(
                                    w_qk_sb[:, 0:1],
                                    w_qk[0:128, :].rearrange("(o p) n -> p o n", p=128))
                                nc.sync.dma_start(xq4[:, 1:4], src_x[:, 1:4])
                                nc.gpsimd.dma_start(
                                    w_qk_sb[:, 1:2],
                                    w_qk[128:256, :].rearrange("(o p) n -> p o n", p=128))
                                nc.gpsimd.dma_start(wv2a[:, 1:2], src_wv[:, 1:2])
                            else:
                                if tcx == 0:
                                    nc.gpsimd.dma_start(
                                        w_qk_sb[:, cg * 4:cg * 4 + 2],
                                        w_qk[cg * 512:cg * 512 + 256, :].rearrange("(o p) n -> p o n", p=128))
                                nc.sync.dma_start(xq4[:], src_x)
                                nc.gpsimd.dma_start(wv2a[:], src_wv)
                            xts.append(xq4)
                            wv2b = wvp.tile([128, 2, DQ], f32r, tag="wv", name=f"wvb_{tcx}_{cg}")
                            nc.gpsimd.dma_start(
                                wv2b[:], w_v[cg * 512 + 256:(cg + 1) * 512, :].rearrange("(o p) n -> p o n", p=128))
                            if tcx == 0:
                                nc.gpsimd.dma_start(
                                    w_qk_sb[:, cg * 4 + 2:cg * 4 + 4],
                                    w_qk[cg * 512 + 256:(cg + 1) * 512, :].rearrange("(o p) n -> p o n", p=128))
                            if tcx == 0 and cg == 1:
                                nc.sync.dma_start(b_qk_sb[:], b_qk)
                                nc.sync.dma_start(b_v_sb[:], b_v)
                            for cs in range(4):
                                ct = cg * 4 + cs
                                wvt = (wv2a if cs < 2 else wv2b)[:, cs % 2]
                                # first tile: V matmuls lead (their operands are
                                # the first 512KB to arrive), QK follows
                                v_first = first and cs == 0
                                if v_first:
                                    for ts_ in range(4):
                                        nc.tensor.matmul(
                                            psv[ts_][:], xq4[:, cs, ts_ * 128:(ts_ + 1) * 128],
                                            wvt, start=(ct == 0), stop=(ct == CT - 1))
                                for blk in range(HL):
                                    nc.tensor.matmul(
                                        psq[blk][:], w_qk_sb[:, ct, blk * 128:(blk + 1) * 128],
                                        xq4[:, cs], start=(ct == 0), stop=(ct == CT - 1))
                                if not v_first:
                                    for ts_ in range(4):
                                        nc.tensor.matmul(
                                            psv[ts_][:], xq4[:, cs, ts_ * 128:(ts_ + 1) * 128],
                                            wvt, start=(ct == 0), stop=(ct == CT - 1))
                        for ts_ in range(4):
                            nc.vector.tensor_tensor(
                                v_sb[:, tcx * 4 + ts_], psv[ts_][:], b_v_sb[:], ADD)
                        for blk in range(HL):
                            nc.scalar.activation(
                                qk_sb[:, blk, tcx * 512:(tcx + 1) * 512], psq[blk][:],
                                Ident, bias=b_qk_sb[:, blk:blk + 1], scale=1.0)
                        # pass B: QK blocks 4-7 over the retained x tiles
                        psqb = []
                        for blk in range(HL, 2 * HL):
                            p = ps8.tile([128, 512], f32, tag="ps512", name=f"psq_{tcx}_{blk}")
                            psqb.append(p)
                        for cg in range(4):
                            for cs in range(4):
                                ct = cg * 4 + cs
                                for blk in range(HL, 2 * HL):
                                    nc.tensor.matmul(
                                        psqb[blk - HL][:], w_qk_sb[:, ct, blk * 128:(blk + 1) * 128],
                                        xts[cg][:, cs], start=(ct == 0), stop=(ct == CT - 1))
                        for blk in range(HL, 2 * HL):
                            nc.scalar.activation(
                                qk_sb[:, blk, tcx * 512:(tcx + 1) * 512], psqb[blk - HL][:],
                                Ident, bias=b_qk_sb[:, blk:blk + 1], scale=1.0)

            # ---------------- Phase ATT ----------------
            with tc.tile_pool(name="attc", bufs=1) as attc, \
                 tc.tile_pool(name="osb", bufs=1) as osbp, \
                 tc.tile_pool(name="wp", bufs=1) as wpp:
                masks_sb = attc.tile([128, 4 * 512], f32r)
                nc.gpsimd.dma_start(masks_sb[:], masks)
                ones_sb = attc.tile([128, 128], f32r)
                nc.gpsimd.dma_start(ones_sb[:], ones_d)
                ident_sb = attc.tile([128, 128], f32r)
                nc.gpsimd.dma_start(ident_sb[:], ident_d)
                o_sb = osbp.tile([128, HL, T], f32r)        # [d, h, t]
                wp_sb = wpp.tile([128, HL, C], f32r)        # loaded after first (h,qc)
                att_ctx = [
                    tc.tile_pool(name="pt", bufs=8),
                    tc.tile_pool(name="rec", bufs=1),
                    tc.tile_pool(name="psS", bufs=2, space="PSUM"),
                    tc.tile_pool(name="psO", bufs=2, space="PSUM"),
                    tc.tile_pool(name="psD", bufs=2, space="PSUM"),
                ]
                from contextlib import ExitStack
                att_stack = ExitStack()
                ptp, recp, psSp, psOp, psDp = [att_stack.enter_context(c) for c in att_ctx]
                for h in range(HL):
                    q_blk = h
                    k_blk = HL + h
                    # qc descending: the first iteration (qc=3) opens with 12
                    # mask-free kk-tiles, hiding the masks/ident DMA latency so
                    # the PE never idles past the HAM re-throttle window.
                    for qc in reversed(range(TC4)):
                        if h == 0 and qc == 2:
                            # cache outputs + W_p preload during the DMA-quiet ATT
                            # phase; gpsimd queue, behind the masks/ones/ident loads
                            nc.gpsimd.dma_start(kT_out.rearrange("(o p) t -> p o t", p=128),
                                                qk_sb[:, HL:2 * HL, :])
                            nc.gpsimd.dma_start(v_out.rearrange("(o p) n -> p o n", p=128), v_sb[:])
                            nc.gpsimd.dma_start(wp_sb[:], w_p)
                        nk = 4 * qc + 4
                        ng = nk // 2
                        pts = []
                        for g in range(ng):
                            psS = psSp.tile([128, 1024], f32, tag="psS", name=f"psS_{h}_{qc}_{g}")
                            for j in range(2):
                                kt = 2 * g + j
                                off = kt - 4 * qc
                                diag = off >= 0
                                nc.tensor.matmul(
                                    psS[:, j * 512:(j + 1) * 512],
                                    qk_sb[:, k_blk, kt * 128:(kt + 1) * 128],
                                    qk_sb[:, q_blk, qc * 512:(qc + 1) * 512],
                                    start=True, stop=not diag)
                                if diag:
                                    # additive causal mask (0 / -1e9) via identity matmul
                                    nc.tensor.matmul(
                                        psS[:, j * 512:(j + 1) * 512], ident_sb[:],
                                        masks_sb[:, off * 512:(off + 1) * 512],
                                        start=False, stop=True)
                            pt = ptp.tile([128, 1024], f32r, tag="pt", name=f"pt_{h}_{qc}_{g}")
                            nc.scalar.activation(pt[:], psS[:], Exp, bias=0.0, scale=SM_SCALE)
                            pts.append(pt)
                        # O^T accumulation and denominators
                        psO = psOp.tile([128, 512], f32, tag="psO", name=f"psO_{h}_{qc}")
                        for kt in range(nk):
                            nc.tensor.matmul(
                                psO[:], v_sb[:, kt, h * 128:(h + 1) * 128],
                                pts[kt // 2][:, (kt % 2) * 512:((kt % 2) + 1) * 512],
                                start=(kt == 0), stop=(kt == nk - 1))
                        psD = psDp.tile([128, 512], f32, tag="psD", name=f"psD_{h}_{qc}")
                        for kt in range(nk):
                            nc.tensor.matmul(
                                psD[:], ones_sb[:],
                                pts[kt // 2][:, (kt % 2) * 512:((kt % 2) + 1) * 512],
                                start=(kt == 0), stop=(kt == nk - 1))
                        rec = recp.tile([128, 512], f32, tag="rec", name=f"rec_{h}_{qc}")
                        nc.vector.reciprocal(rec[:], psD[:])
                        nc.vector.tensor_tensor(
                            o_sb[:, h, qc * 512:(qc + 1) * 512], psO[:], rec[:], MUL)

                att_stack.close()

                # ---------------- Phase PROJ: y_partial = O @ W_p ----------------
                with tc.tile_pool(name="yst", bufs=4) as ystp, \
                     tc.tile_pool(name="psY", bufs=4, space="PSUM") as psYp:
                    for tt in range(TT):
                        for cc in range(TC4):
                            psY = psYp.tile([128, 512], f32, tag="psY", name=f"psY_{tt}_{cc}")
                            for h in range(HL):
                                nc.tensor.matmul(
                                    psY[:], o_sb[:, h, tt * 128:(tt + 1) * 128],
                                    wp_sb[:, h, cc * 512:(cc + 1) * 512],
                                    start=(h == 0), stop=(h == HL - 1))
                            yst = ystp.tile([128, 512], f32, tag="yst", name=f"yst_{tt}_{cc}")
                            nc.any.tensor_copy(yst[:], psY[:])
                            nc.sync.dma_start(
                                y_p[tt * 128:(tt + 1) * 128, cc * 512:(cc + 1) * 512], yst[:])

    nc.compile()
    return nc


def _get_nc():
    if "nc" not in _CACHE:
        _CACHE["nc"] = _build()
    return _CACHE["nc"]


def _make_masks():
    # additive causal masks for the 4 diagonal-band offsets: 0 keep, -1e9 drop
    q = np.arange(512)
    kk = np.arange(128)
    cols = []
    for o in range(4):
        keep = kk[:, None] + o * 128 <= q[None, :]
        cols.append(np.where(keep, 0.0, -1e9).astype(np.float32))
    return np.concatenate(cols, axis=1)  # [128, 2048]


def _make_in_maps(x, W_attn, b_attn, W_proj):
    masks = _make_masks()
    in_maps = []
    for core in range(NCORES):
        b = core // HG
        hg = core % HG
        lo, hi = hg * DQ, (hg + 1) * DQ
        xT_b = np.ascontiguousarray(x[b].T)                                  # [C, T]
        w_qk = np.ascontiguousarray(
            np.concatenate([W_attn[:, lo:hi], W_attn[:, C + lo:C + hi]], axis=1))
        w_v = np.ascontiguousarray(W_attn[:, 2 * C + lo:2 * C + hi])
        w_p = np.ascontiguousarray(
            W_proj[lo:hi, :].reshape(HL, 128, C).transpose(1, 0, 2))         # [128, HL, C]
        b_qk = np.ascontiguousarray(
            np.concatenate([b_attn[lo:hi], b_attn[C + lo:C + hi]]).reshape(2 * HL, 128).T)
        b_v = np.ascontiguousarray(
            np.tile(b_attn[2 * C + lo:2 * C + hi][None, :], (128, 1)))
        in_maps.append({
            "xT": xT_b, "w_qk": w_qk, "w_v": w_v, "w_p": w_p,
            "b_qk": b_qk, "b_v": b_v, "masks": masks,
            "ones_d": np.ones((128, 128), dtype=np.float32),
            "ident_d": np.eye(128, dtype=np.float32),
        })
    return in_maps


def kernel(x, attention_mask, W_attn, b_attn, W_proj, b_proj):
    from concourse import bass_utils

    x = np.asarray(x, dtype=np.float32)
    W_attn = np.asarray(W_attn, dtype=np.float32)
    b_attn = np.asarray(b_attn, dtype=np.float32)
    W_proj = np.asarray(W_proj, dtype=np.float32)
    b_proj = np.asarray(b_proj, dtype=np.float32)

    nc = _get_nc()
    in_maps = _make_in_maps(x, W_attn, b_attn, W_proj)
    res = bass_utils.run_bass_kernel_spmd(nc, in_maps, core_ids=list(range(NCORES)))

    y = np.empty((B, T, C), dtype=np.float32)
    cache = np.empty((B, T, 2 * C), dtype=np.float32)
    for b in range(B):
        acc = None
        for hg in range(HG):
            r = res.results[b * HG + hg]
            acc = r["y_p"].copy() if acc is None else acc + r["y_p"]
            lo, hi = hg * DQ, (hg + 1) * DQ
            cache[b, :, lo:hi] = r["kT_out"].T
            cache[b, :, C + lo:C + hi] = r["v_out"]
        y[b] = acc + b_proj[None, :]
    return (y, cache)
